# revision 34
# baseline (speedup 1.0000x reference)
"""CRF-RNN (dense Gaussian CRF mean-field) Trainium2 kernel, 8 NeuronCores.

Strategy
--------
N = 8*32*32 = 8192 voxels, L = 21 labels, 5 mean-field iterations.
- Column sharding: core r owns z-slice r (1024 voxels = its output columns).
- Bilateral filter via a quadratic-exponential factorization: the kernel
  exponent x = -|f_i - f_j|^2/2 spans only [-0.21, 0] (ALPHA/BETA are large
  vs the feature ranges), so exp(x) ~ c0 + c1 x + c2 x^2 with ~5e-5 abs
  error. x is a rank-8 product u(i).v(j), hence x^2 is rank 36, giving
  K_b ~ PHI @ PSI^T with 45 features. q@K_b collapses to
  R = PHI^T @ q^T (64 tiny matmuls, contraction 8192) followed by
  Mb = R^T @ PSI (one [45]-contraction matmul) - no NxN kernel is ever
  materialized and there is no exp build phase at all.
- The normalizer rides as a "ones" column of the iteration-1 slots; its
  reciprocal is folded into a broadcast multiply of Mb.
- Spatial kernel is separable: q@K_s[:, cols_r] = (sum_z Gz[z,r] q[:,z]) @
  (Gy x Gx); norm_s folded into Kyx columns + the z-mix coefficients.
- Label mixing folded on host: A = C@Ws, B = C@Wb, so
  cur = A @ Sn + B @ Mbn + unary, computed voxel-major with tiny
  [21-contraction, 21] matmuls accumulating straight into the curT PSUM.
- Per-iteration exchange of q blocks ([128,168] fp16) via AllGather through
  DRAM bounce buffers, launched before the own-slice spatial prestart.
"""

import numpy as np

ALPHA, BETA, GAMMA = 160.0, 3.0, 3.0
NUM_ITER = 5
L, D, H, W = 21, 8, 32, 32
NC = 8
NYX = H * W            # 1024
N = D * NYX            # 8192
NT = NYX // 128        # 8 chunks per slice
FB = NT * L            # 168  free width of one q block
LW1 = 22               # iter-1 slot entry width: labels 0..20, ones column 21
FB1 = NT * LW1
NTILE = N // 128       # 64 row tiles
NF = 45                # 1 + 8 + 36 quadratic-exponential features

_CACHE = {}


def _build_nc():
    import concourse.bass as bass
    import concourse.bacc as bacc
    import concourse.mybir as mybir
    import concourse.tile as tile
    import concourse.tile_utils as tile_utils

    try:
        tile_utils.max_sbuf_usage = 204 * 1024
    except Exception:
        pass

    f32 = mybir.dt.float32
    f16 = mybir.dt.float16
    AF = mybir.ActivationFunctionType
    OP = mybir.AluOpType

    nc = bacc.Bacc(None, target_bir_lowering=False, num_devices=NC)

    phi_d = nc.declare_dram_parameter("phi", [128, NTILE * NF], f16, isOutput=False)
    psi_d = nc.declare_dram_parameter("psi", [NF, NYX], f16, isOutput=False)
    kyx_d = nc.declare_dram_parameter("kyx", [128, NT * NYX], f16, isOutput=False)
    unary_d = nc.declare_dram_parameter("unaryt", [128, NC * FB], f32, isOutput=False)
    unown_d = nc.declare_dram_parameter("unown", [128, FB], f32, isOutput=False)
    zco_d = nc.declare_dram_parameter("zcoef", [128, NC], f32, isOutput=False)
    zcoo_d = nc.declare_dram_parameter("zcoo", [128, 1], f32, isOutput=False)
    zcor_d = nc.declare_dram_parameter("zcor", [128, NC], f32, isOutput=False)
    at_d = nc.declare_dram_parameter("at", [L, L], f16, isOutput=False)
    bt_d = nc.declare_dram_parameter("bt", [L, L], f16, isOutput=False)
    ones_d = nc.declare_dram_parameter("ones1", [1, L], f32, isOutput=False)
    out_d = nc.declare_dram_parameter("out", [128, FB], f32, isOutput=True)

    with tile.TileContext(nc) as tc:
        with (
            nc.allow_low_precision(
                reason="fp16 messages; tolerance 2e-2, measured ~1e-4"),
            tc.tile_pool(name="persist", bufs=1) as pp,
            tc.tile_pool(name="epi", bufs=2) as ep,
            tc.tile_pool(name="work", bufs=2) as wp,
            tc.tile_pool(name="dram", bufs=2, space="DRAM") as dp,
            tc.tile_pool(name="ps_bil", bufs=1, space="PSUM") as ps_bil,
            tc.tile_pool(name="ps_dmy", bufs=1, space="PSUM") as ps_dmy_p,
        ):
            # ---------------- persistent SBUF ----------------
            sb_phi = pp.tile([128, NTILE * NF], f16, tag="phi")      # 5.6KB/p
            sb_psi = pp.tile([NF, NYX], f16, tag="psi")
            sb_kyx = pp.tile([128, NT * NYX], f16, tag="kyx")        # 16KB/p
            sb_unary = pp.tile([128, NC * FB], f32, tag="unary")     # 5.25KB/p
            sb_unown = pp.tile([128, FB], f32, tag="unown")
            sb_s1 = pp.tile([128, NC * FB1], f16, tag="s1")
            sb_slots = [pp.tile([128, NC * FB], f16, tag=f"slots{j % 2}",
                                name=f"sb_slots{j}")
                        for j in range(NUM_ITER - 1)]                # ping-pong
            sb_zco = pp.tile([128, NC], f32, tag="zco")
            sb_zcoo = pp.tile([128, 1], f32, tag="zcoo")
            sb_zcor = pp.tile([128, NC], f32, tag="zcor")
            sb_at = pp.tile([L, L], f16, tag="at")
            sb_bt = pp.tile([L, L], f16, tag="bt")
            sb_ones = pp.tile([1, L], f32, tag="ones")
            sb_recipb = pp.tile([L, NYX], f32, tag="recipb")
            sb_nrow = pp.tile([1, NYX], f16, tag="nrow")
            sb_nrcp = pp.tile([1, NYX], f32, tag="nrcp")
            sb_out = pp.tile([128, FB], f32, tag="outt")

            # ---------------- input DMAs ----------------
            # warmup-collective input first (absorbs inter-core start skew
            # as early as possible), then unary: the iteration-1 softmax is
            # the startup critical path now that there is no build phase.
            wu_in = dp.tile([128, 8], f16, tag="wuin")
            wu_sb = pp.tile([128, 8], f16, tag="wusb")
            nc.vector.memset(wu_sb[:, :], 0.0)
            nc.sync.dma_start(wu_in[:, :], wu_sb[:, :])
            nc.sync.dma_start(sb_phi[:, :], phi_d[:, :])
            nc.sync.dma_start(sb_psi[:, :], psi_d[:, :])
            for s in range(NC):
                nc.sync.dma_start(sb_unary[:, s * FB:(s + 1) * FB],
                                  unary_d[:, s * FB:(s + 1) * FB])
            nc.sync.dma_start(sb_zco[:, :], zco_d[:, :])
            nc.sync.dma_start(sb_zcoo[:, :], zcoo_d[:, :])
            nc.sync.dma_start(sb_zcor[:, :], zcor_d[:, :])
            nc.sync.dma_start(sb_at[:, :], at_d[:, :])
            nc.sync.dma_start(sb_bt[:, :], bt_d[:, :])
            nc.sync.dma_start(sb_ones[:, :], ones_d[:, :])
            nc.sync.dma_start(sb_unown[:, :], unown_d[:, :])

            # warm up the collective path early
            wu_out = dp.tile([128 * NC, 8], f16, tag="wuout")
            nc.gpsimd.collective_compute(
                "AllGather", mybir.AluOpType.bypass,
                replica_groups=[list(range(NC))],
                ins=[wu_in.opt()], outs=[wu_out.opt()],
            )

            nc.sync.dma_start(sb_kyx[:, :], kyx_d[:, :])

            # ---------------- iter-1 softmax for all 8 slices ----------------
            # exp buffer aliases sb_slots[0]: fully consumed before the
            # iter-0 exchange writes slots[0] (WAR serialized by the tile
            # framework).
            sb_exp1 = sb_slots[0]
            sb_red1 = pp.tile([128, NC * NT], f32, tag="red1")
            sb_rcp1 = pp.tile([128, NC * NT], f32, tag="rcp1")
            s1_v = sb_s1[:, :].rearrange("p (g l) -> p g l", l=LW1)
            nc.vector.memset(sb_s1[:, :], 0.0)
            nc.vector.memset(s1_v[:, :, LW1 - 1:LW1], 1.0)
            un_v = sb_exp1[:, :].rearrange("p (g l) -> p g l", l=L)
            for s in range(NC):
                sl = slice(s * FB, (s + 1) * FB)
                gl = slice(s * NT, (s + 1) * NT)
                nc.scalar.activation(sb_exp1[:, sl], sb_unary[:, sl], AF.Exp)
                nc.vector.tensor_reduce(
                    sb_red1[:, gl], un_v[:, gl, :],
                    mybir.AxisListType.X, OP.add)
                nc.vector.reciprocal(sb_rcp1[:, gl], sb_red1[:, gl])
                nc.vector.tensor_tensor(
                    s1_v[:, gl, 0:L], un_v[:, gl, :],
                    sb_rcp1[:, gl].broadcast_to([128, NT, L]),
                    OP.mult,
                )

            phi_v = sb_phi[:, :].rearrange("p (n m) -> p n m", m=NF)

            with tc.tile_pool(name="ps_rest", bufs=1, space="PSUM") as ps_r:
                # ================= iterations =================
                slots_l = sb_s1[:, :].rearrange("p (n l) -> p n l", l=LW1)
                slots_dt = sb_s1[:, :].rearrange(
                    "p (d t l) -> p d t l", d=NC, l=LW1)

                for it in range(NUM_ITER):
                    last = it == NUM_ITER - 1
                    rows = LW1 if it == 0 else L   # iter 0 carries the ones col

                    # ---- bilateral: R = PHI^T q^T, then Mb = R^T PSI ----
                    # two concurrent PE column groups halve the stream time;
                    # group 1 lands at PSUM partitions 64+.
                    ps_rr = ps_r.tile([128, LW1], f32, tag="r45",
                                      name=f"ps_rr{it}")
                    for k in range(32):
                        for g in range(2):
                            dt = 32 * g + k
                            nc.tensor.matmul(
                                ps_rr[64 * g:64 * g + NF, 0:rows],
                                phi_v[:, dt, :],
                                slots_l[:, dt, 0:rows],
                                start=(k == 0), stop=(k % 16 == 15),
                                skip_group_check=True,
                                tile_position=(0, 64 * g),
                            )
                    # a few keep-warm matmuls so the PE does not drop
                    # its DVFS state while the R copies round-trip
                    ps_dmy0 = ps_dmy_p.tile([1, 512], f32, tag="dmy",
                                            name=f"ps_dmy0_{it}")
                    for _w in range(5):
                        nc.tensor.matmul(
                            ps_dmy0[0:1, 0:512], sb_at[0:1, 0:1],
                            sb_kyx[0:1, 0:512],
                            start=True, stop=True, skip_group_check=True)
                    rsb = wp.tile([NF, LW1], f16, tag="rsb")
                    rsb_b = wp.tile([NF, LW1], f16, tag="rsbb")
                    nc.scalar.copy(rsb[:, 0:rows], ps_rr[0:NF, 0:rows])
                    nc.scalar.copy(rsb_b[:, 0:rows], ps_rr[64:64 + NF, 0:rows])
                    # the two R halves sum in the PSUM accumulation of the
                    # assembly matmuls - no cross-engine add on the path
                    cur_bil = ps_bil.tile([LW1, NYX], f32, tag="bil")
                    for h in range(2):
                        for gi, rs in enumerate((rsb, rsb_b)):
                            nc.tensor.matmul(
                                cur_bil[0:rows, h * 512:(h + 1) * 512],
                                rs[:, 0:rows],
                                sb_psi[:, h * 512:(h + 1) * 512],
                                start=(gi == 0), stop=(gi == 1),
                                skip_group_check=True,
                            )

                    if it == 0:
                        # norm row -> SBUF (base-21 partition needs the copy
                        # + DMA detour), reciprocal, broadcast to 21 rows
                        u22 = ep.tile([LW1, NYX], f16, tag="epi")
                        nc.scalar.copy(u22[:, :], cur_bil[0:LW1, :])
                        nc.sync.dma_start(sb_nrow[:, :], u22[L:LW1, :])
                        nc.vector.reciprocal(sb_nrcp[:, :], sb_nrow[:, :])
                        ps_nb = ps_r.tile([L, NYX], f32, tag="spat")
                        for h in range(2):
                            nc.tensor.matmul(
                                ps_nb[:, h * 512:(h + 1) * 512], sb_ones[:, :],
                                sb_nrcp[:, h * 512:(h + 1) * 512],
                                start=True, stop=True,
                            )
                        nc.scalar.copy(sb_recipb[:, :], ps_nb[:, :])
                    mbn = ep.tile([L, NYX], f16, tag="epi16")
                    nc.vector.tensor_tensor(
                        mbn[:, :], cur_bil[0:L, :], sb_recipb[:, :], OP.mult)

                    # ---- spatial: z-mix then yx filter ----
                    # iterations >=1: the own-slice part was pre-started from
                    # qblk before the AllGather; add the other 7 slices here.
                    kyx_v = sb_kyx[:, :].rearrange("p (k c) -> p k c", c=NYX)
                    zmix = sb_zco if it == 0 else sb_zcor
                    brt = wp.tile([128, FB], f16, tag="brt")
                    nc.vector.tensor_scalar_mul(
                        brt[:, :], slots_dt[:, 0, :, 0:L], zmix[:, 0:1])
                    for d in range(1, NC):
                        nc.vector.scalar_tensor_tensor(
                            brt[:, :], slots_dt[:, d, :, 0:L], zmix[:, d:d + 1],
                            brt[:, :], OP.mult, OP.add)
                    brt_v = brt[:, :].rearrange("p (t l) -> p t l", l=L)
                    if it == 0:
                        ps_sp = ps_r.tile([L, NYX], f32, tag="spat")
                    else:
                        ps_sp = ps_sp_pending
                    for k in range(NT):
                        for h in range(2):
                            nc.tensor.matmul(
                                ps_sp[:, h * 512:(h + 1) * 512],
                                brt_v[:, k, :],
                                kyx_v[:, k, h * 512:(h + 1) * 512],
                                start=(k == 0 and it == 0),
                                stop=(k == NT - 1),
                                skip_group_check=True,
                            )
                    sn = ep.tile([L, NYX], f16, tag="epi16")
                    nc.scalar.copy(sn[:, :], ps_sp[:, :])

                    # ---- curT = Sn^T@A^T + Mbn^T@B^T (+ unary), voxel-major --
                    ps_ct = ps_r.tile([128, FB], f32, tag="curt")
                    for tl in range(NT):
                        nc.tensor.matmul(
                            ps_ct[:, tl * L:(tl + 1) * L],
                            sn[:, tl * 128:(tl + 1) * 128],
                            sb_at[:, :], start=True, stop=False,
                            skip_group_check=True)
                        nc.tensor.matmul(
                            ps_ct[:, tl * L:(tl + 1) * L],
                            mbn[:, tl * 128:(tl + 1) * 128],
                            sb_bt[:, :], start=False, stop=True,
                            skip_group_check=True)
                    sm = wp.tile([128, FB], f32, tag="sum")
                    nc.vector.tensor_tensor(
                        sm[:, :], ps_ct[:, :], sb_unown[:, :], OP.add)

                    # ---- softmax over labels (free dim) ----
                    ex = wp.tile([128, FB], f32, tag="exp")
                    nc.scalar.activation(ex[:, :], sm[:, :], AF.Exp)
                    ex_v = ex[:, :].rearrange("p (t l) -> p t l", l=L)
                    rd = wp.tile([128, NT], f32, tag="red")
                    nc.vector.tensor_reduce(
                        rd[:, :], ex_v, mybir.AxisListType.X, OP.add)
                    rc = wp.tile([128, NT], f32, tag="rcp")
                    nc.vector.reciprocal(rc[:, :], rd[:, :])
                    if last:
                        nc.vector.tensor_tensor(
                            sb_out[:, :].rearrange("p (t l) -> p t l", l=L),
                            ex_v, rc[:, :].broadcast_to([128, NT, L]), OP.mult)
                        nc.sync.dma_start(out_d[:, :], sb_out[:, :])
                    else:
                        qblk = wp.tile([128, FB], f16, tag="qblk")
                        nc.vector.tensor_tensor(
                            qblk[:, :].rearrange("p (t l) -> p t l", l=L),
                            ex_v, rc[:, :].broadcast_to([128, NT, L]), OP.mult)

                        # ---- exchange first: AllGather of the q blocks ----
                        cc_in = dp.tile([128, FB], f16, tag="ccin")
                        cc_out = dp.tile([128 * NC, FB], f16, tag="ccout")
                        nc.sync.dma_start(cc_in[:, :], qblk[:, :])
                        nc.gpsimd.collective_compute(
                            "AllGather",
                            mybir.AluOpType.bypass,
                            replica_groups=[list(range(NC))],
                            ins=[cc_in.opt()],
                            outs=[cc_out.opt()],
                        )
                        nxt = sb_slots[it]
                        nc.sync.dma_start(
                            nxt[:, :].rearrange("p (d f) -> p d f", d=NC),
                            cc_out[:, :].rearrange("(d p) f -> p d f", p=128),
                        )

                        # pre-start next iteration's spatial own-slice part
                        ps_sp_pending = ps_r.tile([L, NYX], f32, tag="spat",
                                                  name=f"ps_spp{it}")
                        bo = wp.tile([128, FB], f16, tag="brto")
                        nc.vector.tensor_scalar_mul(
                            bo[:, :], qblk[:, :], sb_zcoo[:, 0:1])
                        bo_v = bo[:, :].rearrange("p (t l) -> p t l", l=L)
                        for k in range(NT):
                            for h in range(2):
                                nc.tensor.matmul(
                                    ps_sp_pending[:, h * 512:(h + 1) * 512],
                                    bo_v[:, k, :],
                                    kyx_v[:, k, h * 512:(h + 1) * 512],
                                    start=(k == 0), stop=False,
                                    skip_group_check=True,
                                )

                        ps_dmy = ps_dmy_p.tile([1, 512], f32, tag="dmy",
                                               name=f"ps_dmy{it}")
                        for _w in range(20):
                            nc.tensor.matmul(
                                ps_dmy[0:1, 0:512], sb_at[0:1, 0:1],
                                sb_kyx[0:1, 0:512],
                                start=True, stop=True,
                                skip_group_check=True,
                            )

                        slots_l = nxt[:, :].rearrange("p (n l) -> p n l", l=L)
                        slots_dt = nxt[:, :].rearrange(
                            "p (d t l) -> p d t l", d=NC, l=L)
    nc.compile()
    return nc


def _host_prep(image, logits):
    """Per-core input dicts (global voxel order). Returns list of 8 dicts."""
    img = np.asarray(image, dtype=np.float32)[0]      # [3, D, H, W]
    lg = np.asarray(logits, dtype=np.float32)[0]      # [L, D, H, W]

    zz, yy, xx = np.meshgrid(
        np.arange(D), np.arange(H), np.arange(W), indexing="ij")
    pos = np.stack([zz, yy, xx], -1).reshape(N, 3).astype(np.float64)
    rgb = img.reshape(3, N).T.astype(np.float64)
    feat = np.concatenate([pos / ALPHA, rgb / BETA], axis=1)   # [N, 6]
    sq = np.sum(feat * feat, axis=1)                           # [N]

    # quadratic fit of exp(x) over the exponent range [-xmax, 0]
    dmax = 2.0 * (31.0 / ALPHA) ** 2 + (7.0 / ALPHA) ** 2 + np.sum(
        ((rgb.max(0) - rgb.min(0)) / BETA) ** 2)
    xmax = 0.5 * dmax
    xs = np.linspace(-xmax, 0.0, 4001)
    c2, c1, c0 = np.polyfit(xs, np.exp(xs), 2)

    # u(i).v(j) = f_i.f_j - |f_i|^2/2 - |f_j|^2/2 = -|f_i - f_j|^2/2
    u = np.concatenate([feat, -0.5 * sq[:, None], np.ones((N, 1))], axis=1)
    v = np.concatenate([feat, np.ones((N, 1)), -0.5 * sq[:, None]], axis=1)
    cols = []
    pcols = []
    cols.append(np.ones(N)); pcols.append(np.full(N, c0))
    for m in range(8):
        cols.append(c1 * u[:, m]); pcols.append(v[:, m])
    for m in range(8):
        for m2 in range(m, 8):
            w = 2.0 if m2 > m else 1.0
            cols.append(w * c2 * u[:, m] * u[:, m2])
            pcols.append(v[:, m] * v[:, m2])
    PHI = np.stack(cols, axis=1).astype(np.float16)    # [N, 45]
    PSI = np.stack(pcols, axis=1).astype(np.float16)   # [N, 45]

    r1 = np.arange(D, dtype=np.float32)
    Gz = np.exp(-0.5 * ((r1[:, None] - r1[None, :]) / GAMMA) ** 2)
    r2 = np.arange(H, dtype=np.float32)
    Gy = np.exp(-0.5 * ((r2[:, None] - r2[None, :]) / GAMMA) ** 2)
    Kyx = np.kron(Gy, Gy).astype(np.float32)          # H == W so Gy == Gx
    nyx = Kyx.sum(axis=0)
    Kyx_n = (Kyx / nyx[None, :]).astype(np.float16)   # [1024, 1024]
    czsum = Gz.sum(axis=0)

    unary = lg.reshape(L, N)
    # voxel-major: blkT[p, s, t*L + l] = unary[l, s*NYX + t*128 + p]
    blkT = unary.reshape(L, D, NT, 128).transpose(3, 1, 2, 0)  # [128, D, NT, L]
    un = np.ascontiguousarray(blkT.reshape(128, NC * FB))

    phi_in = np.ascontiguousarray(
        PHI.reshape(NTILE, 128, NF).transpose(1, 0, 2).reshape(128, NTILE * NF))
    kyx_in = np.ascontiguousarray(
        Kyx_n.reshape(NT, 128, NYX).transpose(1, 0, 2).reshape(128, NT * NYX))

    maps = []
    for r in range(NC):
        psi_r = np.ascontiguousarray(PSI[r * NYX:(r + 1) * NYX].T)  # [45,1024]
        zvec = (Gz[:, r] / czsum[r]).astype(np.float32)
        zco = np.tile(zvec, (128, 1))
        zrest = zvec.copy(); zrest[r] = 0.0
        unown = np.ascontiguousarray(blkT[:, r].reshape(128, FB))
        maps.append({
            "zcoo": np.full((128, 1), zvec[r], np.float32),
            "zcor": np.ascontiguousarray(np.tile(zrest, (128, 1))),
            "phi": phi_in,
            "psi": psi_r,
            "kyx": kyx_in,
            "unaryt": un,
            "unown": unown,
            "zcoef": np.ascontiguousarray(zco),
        })
    return maps


def kernel(image, logits, spatial_ker_weights, bilateral_ker_weights,
           compatibility_matrix):
    from concourse.bass_utils import run_bass_kernel_spmd

    if "nc" not in _CACHE:
        _CACHE["nc"] = _build_nc()
    nc = _CACHE["nc"]

    maps = _host_prep(image, logits)
    ws = np.asarray(spatial_ker_weights, np.float64)
    wb = np.asarray(bilateral_ker_weights, np.float64)
    cm = np.asarray(compatibility_matrix, np.float64)
    at = np.ascontiguousarray((cm @ ws).T.astype(np.float16))
    bt = np.ascontiguousarray((cm @ wb).T.astype(np.float16))
    ones1 = np.ones((1, L), np.float32)
    for m in maps:
        m["at"] = at
        m["bt"] = bt
        m["ones1"] = ones1

    res = run_bass_kernel_spmd(nc, maps, core_ids=list(range(NC)))

    out = np.empty((L, D, H, W), dtype=np.float32)
    for r in range(NC):
        blk = res.results[r]["out"]                   # [128, 168]
        out[:, r] = blk.reshape(128, NT, L).transpose(2, 1, 0).reshape(L, H, W)
    return out[None]


# revision 37
# speedup vs baseline: 1.0195x; 1.0195x over previous
"""CRF-RNN (dense Gaussian CRF mean-field) Trainium2 kernel, 8 NeuronCores.

Strategy
--------
N = 8*32*32 = 8192 voxels, L = 21 labels, 5 mean-field iterations.
- Column sharding: core r owns z-slice r (1024 voxels = its output columns).
- Bilateral filter via a quadratic-exponential factorization: the kernel
  exponent x = -|f_i - f_j|^2/2 spans only [-0.21, 0] (ALPHA/BETA are large
  vs the feature ranges), so exp(x) ~ c0 + c1 x + c2 x^2 with ~5e-5 abs
  error. x is a rank-8 product u(i).v(j), hence x^2 is rank 36, giving
  K_b ~ PHI @ PSI^T with 45 features. q@K_b collapses to
  R = PHI^T @ q^T (64 tiny matmuls, contraction 8192) followed by
  Mb = R^T @ PSI (one [45]-contraction matmul) - no NxN kernel is ever
  materialized and there is no exp build phase at all.
- The normalizer rides as a "ones" column of the iteration-1 slots; its
  reciprocal is folded into a broadcast multiply of Mb.
- Spatial kernel is separable: q@K_s[:, cols_r] = (sum_z Gz[z,r] q[:,z]) @
  (Gy x Gx); norm_s folded into Kyx columns + the z-mix coefficients.
- Label mixing folded on host: A = C@Ws, B = C@Wb, so
  cur = A @ Sn + B @ Mbn + unary, computed voxel-major with tiny
  [21-contraction, 21] matmuls accumulating straight into the curT PSUM.
- Per-iteration exchange of q blocks ([128,168] fp16) via AllGather through
  DRAM bounce buffers, launched before the own-slice spatial prestart.
"""

import numpy as np

ALPHA, BETA, GAMMA = 160.0, 3.0, 3.0
NUM_ITER = 5
L, D, H, W = 21, 8, 32, 32
NC = 8
NYX = H * W            # 1024
N = D * NYX            # 8192
NT = NYX // 128        # 8 chunks per slice
FB = NT * L            # 168  free width of one q block
LW1 = 22               # iter-1 slot entry width: labels 0..20, ones column 21
FB1 = NT * LW1
NTILE = N // 128       # 64 row tiles
NF = 45                # 1 + 8 + 36 quadratic-exponential features

_CACHE = {}


def _build_nc():
    import concourse.bass as bass
    import concourse.bacc as bacc
    import concourse.mybir as mybir
    import concourse.tile as tile
    import concourse.tile_utils as tile_utils

    try:
        tile_utils.max_sbuf_usage = 204 * 1024
    except Exception:
        pass

    f32 = mybir.dt.float32
    f16 = mybir.dt.float16
    AF = mybir.ActivationFunctionType
    OP = mybir.AluOpType

    nc = bacc.Bacc(None, target_bir_lowering=False, num_devices=NC)

    phi_d = nc.declare_dram_parameter("phi", [128, NTILE * NF], f16, isOutput=False)
    psi_d = nc.declare_dram_parameter("psi", [NF, NYX], f16, isOutput=False)
    kyx_d = nc.declare_dram_parameter("kyx", [128, NT * NYX], f16, isOutput=False)
    unary_d = nc.declare_dram_parameter("unaryt", [128, NC * FB], f32, isOutput=False)
    unown_d = nc.declare_dram_parameter("unown", [128, FB], f32, isOutput=False)
    zco_d = nc.declare_dram_parameter("zcoef", [128, NC], f32, isOutput=False)
    zcoo_d = nc.declare_dram_parameter("zcoo", [128, 1], f32, isOutput=False)
    zcor_d = nc.declare_dram_parameter("zcor", [128, NC], f32, isOutput=False)
    at_d = nc.declare_dram_parameter("at", [L, L], f16, isOutput=False)
    bt_d = nc.declare_dram_parameter("bt", [L, L], f16, isOutput=False)
    ones_d = nc.declare_dram_parameter("ones1", [1, L], f32, isOutput=False)
    out_d = nc.declare_dram_parameter("out", [128, FB], f32, isOutput=True)

    with tile.TileContext(nc) as tc:
        with (
            nc.allow_low_precision(
                reason="fp16 messages; tolerance 2e-2, measured ~1e-4"),
            tc.tile_pool(name="persist", bufs=1) as pp,
            tc.tile_pool(name="epi", bufs=2) as ep,
            tc.tile_pool(name="work", bufs=2) as wp,
            tc.tile_pool(name="dram", bufs=2, space="DRAM") as dp,
            tc.tile_pool(name="ps_bil", bufs=1, space="PSUM") as ps_bil,
            tc.tile_pool(name="ps_dmy", bufs=1, space="PSUM") as ps_dmy_p,
        ):
            # ---------------- persistent SBUF ----------------
            sb_phi = pp.tile([128, NTILE * NF], f16, tag="phi")      # 5.6KB/p
            sb_psi = pp.tile([NF, NYX], f16, tag="psi")
            sb_kyx = pp.tile([128, NT * NYX], f16, tag="kyx")        # 16KB/p
            sb_unary = pp.tile([128, NC * FB], f32, tag="unary")     # 5.25KB/p
            sb_unown = pp.tile([128, FB], f32, tag="unown")
            sb_s1 = pp.tile([128, NC * FB1], f16, tag="s1")
            sb_slots = [pp.tile([128, NC * FB], f16, tag=f"slots{j % 2}",
                                name=f"sb_slots{j}")
                        for j in range(NUM_ITER - 1)]                # ping-pong
            sb_zco = pp.tile([128, NC], f32, tag="zco")
            sb_zcoo = pp.tile([128, 1], f32, tag="zcoo")
            sb_zcor = pp.tile([128, NC], f32, tag="zcor")
            sb_at = pp.tile([L, L], f16, tag="at")
            sb_bt = pp.tile([L, L], f16, tag="bt")
            sb_ones = pp.tile([1, L], f32, tag="ones")
            sb_recipb = pp.tile([L, NYX], f32, tag="recipb")
            sb_nrow = pp.tile([1, NYX], f16, tag="nrow")
            sb_nrcp = pp.tile([1, NYX], f32, tag="nrcp")
            sb_out = pp.tile([128, FB], f32, tag="outt")

            # ---------------- input DMAs ----------------
            # warmup-collective input first (absorbs inter-core start skew
            # as early as possible), then unary: the iteration-1 softmax is
            # the startup critical path now that there is no build phase.
            wu_in = dp.tile([128, 8], f16, tag="wuin")
            wu_sb = pp.tile([128, 8], f16, tag="wusb")
            nc.vector.memset(wu_sb[:, :], 0.0)
            nc.sync.dma_start(wu_in[:, :], wu_sb[:, :])
            nc.sync.dma_start(sb_phi[:, :], phi_d[:, :])
            nc.sync.dma_start(sb_psi[:, :], psi_d[:, :])
            for s in range(NC):
                nc.sync.dma_start(sb_unary[:, s * FB:(s + 1) * FB],
                                  unary_d[:, s * FB:(s + 1) * FB])
            nc.sync.dma_start(sb_zco[:, :], zco_d[:, :])
            nc.sync.dma_start(sb_zcoo[:, :], zcoo_d[:, :])
            nc.sync.dma_start(sb_zcor[:, :], zcor_d[:, :])
            nc.sync.dma_start(sb_at[:, :], at_d[:, :])
            nc.sync.dma_start(sb_bt[:, :], bt_d[:, :])
            nc.sync.dma_start(sb_ones[:, :], ones_d[:, :])
            nc.sync.dma_start(sb_unown[:, :], unown_d[:, :])

            # warm up the collective path early
            wu_out = dp.tile([128 * NC, 8], f16, tag="wuout")
            nc.gpsimd.collective_compute(
                "AllGather", mybir.AluOpType.bypass,
                replica_groups=[list(range(NC))],
                ins=[wu_in.opt()], outs=[wu_out.opt()],
            )

            nc.sync.dma_start(sb_kyx[:, :], kyx_d[:, :])

            # ---------------- iter-1 softmax for all 8 slices ----------------
            # exp buffer aliases sb_slots[0]: fully consumed before the
            # iter-0 exchange writes slots[0] (WAR serialized by the tile
            # framework).
            sb_exp1 = sb_slots[0]
            sb_red1 = pp.tile([128, NC * NT], f32, tag="red1")
            sb_rcp1 = pp.tile([128, NC * NT], f32, tag="rcp1")
            s1_v = sb_s1[:, :].rearrange("p (g l) -> p g l", l=LW1)
            nc.vector.memset(sb_s1[:, :], 0.0)
            nc.vector.memset(s1_v[:, :, LW1 - 1:LW1], 1.0)
            un_v = sb_exp1[:, :].rearrange("p (g l) -> p g l", l=L)
            for s in range(NC):
                sl = slice(s * FB, (s + 1) * FB)
                gl = slice(s * NT, (s + 1) * NT)
                nc.scalar.activation(sb_exp1[:, sl], sb_unary[:, sl], AF.Exp)
                nc.vector.tensor_reduce(
                    sb_red1[:, gl], un_v[:, gl, :],
                    mybir.AxisListType.X, OP.add)
                nc.vector.reciprocal(sb_rcp1[:, gl], sb_red1[:, gl])
                nc.vector.tensor_tensor(
                    s1_v[:, gl, 0:L], un_v[:, gl, :],
                    sb_rcp1[:, gl].broadcast_to([128, NT, L]),
                    OP.mult,
                )

            phi_v = sb_phi[:, :].rearrange("p (n m) -> p n m", m=NF)

            with tc.tile_pool(name="ps_rest", bufs=1, space="PSUM") as ps_r:
                # ================= iterations =================
                slots_l = sb_s1[:, :].rearrange("p (n l) -> p n l", l=LW1)
                slots_dt = sb_s1[:, :].rearrange(
                    "p (d t l) -> p d t l", d=NC, l=LW1)

                for it in range(NUM_ITER):
                    last = it == NUM_ITER - 1
                    rows = LW1 if it == 0 else L   # iter 0 carries the ones col

                    # ---- bilateral: R = PHI^T q^T, then Mb = R^T PSI ----
                    # two concurrent PE column groups halve the stream time;
                    # group 1 lands at PSUM partitions 64+.
                    ps_rr = ps_r.tile([128, LW1], f32, tag="r45",
                                      name=f"ps_rr{it}")
                    for k in range(32):
                        for g in range(2):
                            dt = 32 * g + k
                            nc.tensor.matmul(
                                ps_rr[64 * g:64 * g + NF, 0:rows],
                                phi_v[:, dt, :],
                                slots_l[:, dt, 0:rows],
                                start=(k == 0), stop=(k % 16 == 15),
                                skip_group_check=True,
                                tile_position=(0, 64 * g),
                            )
                    # a few keep-warm matmuls so the PE does not drop
                    # its DVFS state while the R copies round-trip
                    ps_dmy0 = ps_dmy_p.tile([1, 512], f32, tag="dmy",
                                            name=f"ps_dmy0_{it}")
                    for _w in range(5):
                        nc.tensor.matmul(
                            ps_dmy0[0:1, 0:512], sb_at[0:1, 0:1],
                            sb_kyx[0:1, 0:512],
                            start=True, stop=True, skip_group_check=True)
                    rsb = wp.tile([NF, LW1], f16, tag="rsb")
                    rsb_b = wp.tile([NF, LW1], f16, tag="rsbb")
                    nc.scalar.copy(rsb[:, 0:rows], ps_rr[0:NF, 0:rows])
                    nc.scalar.copy(rsb_b[:, 0:rows], ps_rr[64:64 + NF, 0:rows])
                    # the two R halves sum in the PSUM accumulation of the
                    # assembly matmuls - no cross-engine add on the path
                    cur_bil = ps_bil.tile([LW1, NYX], f32, tag="bil")
                    for h in range(2):
                        for gi, rs in enumerate((rsb, rsb_b)):
                            nc.tensor.matmul(
                                cur_bil[0:rows, h * 512:(h + 1) * 512],
                                rs[:, 0:rows],
                                sb_psi[:, h * 512:(h + 1) * 512],
                                start=(gi == 0), stop=(gi == 1),
                                skip_group_check=True,
                            )

                    if it == 0:
                        # norm row -> SBUF (base-21 partition needs the copy
                        # + DMA detour), reciprocal, broadcast to 21 rows
                        u22 = ep.tile([LW1, NYX], f16, tag="epi")
                        nc.scalar.copy(u22[:, :], cur_bil[0:LW1, :])
                        nc.sync.dma_start(sb_nrow[:, :], u22[L:LW1, :])
                        nc.vector.reciprocal(sb_nrcp[:, :], sb_nrow[:, :])
                        ps_nb = ps_r.tile([L, NYX], f32, tag="spat")
                        for h in range(2):
                            nc.tensor.matmul(
                                ps_nb[:, h * 512:(h + 1) * 512], sb_ones[:, :],
                                sb_nrcp[:, h * 512:(h + 1) * 512],
                                start=True, stop=True,
                            )
                        nc.scalar.copy(sb_recipb[:, :], ps_nb[:, :])
                    mbn = ep.tile([L, NYX], f16, tag="epi16")
                    nc.vector.tensor_tensor(
                        mbn[:, :], cur_bil[0:L, :], sb_recipb[:, :], OP.mult)

                    # ---- spatial: z-mix then yx filter ----
                    # iterations >=1: the own-slice part was pre-started from
                    # qblk before the AllGather; add the other 7 slices here.
                    kyx_v = sb_kyx[:, :].rearrange("p (k c) -> p k c", c=NYX)
                    zmix = sb_zco if it == 0 else sb_zcor
                    brt = wp.tile([128, FB], f16, tag="brt")
                    nc.vector.tensor_scalar_mul(
                        brt[:, :], slots_dt[:, 0, :, 0:L], zmix[:, 0:1])
                    for d in range(1, NC):
                        nc.vector.scalar_tensor_tensor(
                            brt[:, :], slots_dt[:, d, :, 0:L], zmix[:, d:d + 1],
                            brt[:, :], OP.mult, OP.add)
                    brt_v = brt[:, :].rearrange("p (t l) -> p t l", l=L)
                    if it == 0:
                        ps_sp = ps_r.tile([L, NYX], f32, tag="spat")
                    else:
                        ps_sp = ps_sp_pending
                    for k in range(NT):
                        for h in range(2):
                            nc.tensor.matmul(
                                ps_sp[:, h * 512:(h + 1) * 512],
                                brt_v[:, k, :],
                                kyx_v[:, k, h * 512:(h + 1) * 512],
                                start=(k == 0 and it == 0),
                                stop=(k == NT - 1),
                                skip_group_check=True,
                            )
                    sn = ep.tile([L, NYX], f16, tag="epi16")
                    nc.scalar.copy(sn[:, :], ps_sp[:, :])

                    # ---- curT = Sn^T@A^T + Mbn^T@B^T (+ unary), voxel-major --
                    ps_ct = ps_r.tile([128, FB], f32, tag="curt")
                    for tl in range(NT):
                        nc.tensor.matmul(
                            ps_ct[:, tl * L:(tl + 1) * L],
                            sn[:, tl * 128:(tl + 1) * 128],
                            sb_at[:, :], start=True, stop=False,
                            skip_group_check=True)
                        nc.tensor.matmul(
                            ps_ct[:, tl * L:(tl + 1) * L],
                            mbn[:, tl * 128:(tl + 1) * 128],
                            sb_bt[:, :], start=False, stop=True,
                            skip_group_check=True)
                    sm = wp.tile([128, FB], f32, tag="sum")
                    nc.vector.tensor_tensor(
                        sm[:, :], ps_ct[:, :], sb_unown[:, :], OP.add)

                    # ---- softmax over labels (free dim) ----
                    ex = wp.tile([128, FB], f32, tag="exp")
                    nc.scalar.activation(ex[:, :], sm[:, :], AF.Exp)
                    ex_v = ex[:, :].rearrange("p (t l) -> p t l", l=L)
                    rd = wp.tile([128, NT], f32, tag="red")
                    nc.vector.tensor_reduce(
                        rd[:, :], ex_v, mybir.AxisListType.X, OP.add)
                    rc = wp.tile([128, NT], f32, tag="rcp")
                    nc.vector.reciprocal(rc[:, :], rd[:, :])
                    if last:
                        nc.vector.tensor_tensor(
                            sb_out[:, :].rearrange("p (t l) -> p t l", l=L),
                            ex_v, rc[:, :].broadcast_to([128, NT, L]), OP.mult)
                        nc.sync.dma_start(out_d[:, :], sb_out[:, :])
                    else:
                        qblk = wp.tile([128, FB], f16, tag="qblk")
                        nc.vector.tensor_tensor(
                            qblk[:, :].rearrange("p (t l) -> p t l", l=L),
                            ex_v, rc[:, :].broadcast_to([128, NT, L]), OP.mult)

                        # ---- exchange first: AllGather of the q blocks ----
                        cc_in = dp.tile([128, FB], f16, tag="ccin")
                        cc_out = dp.tile([128 * NC, FB], f16, tag="ccout")
                        nc.sync.dma_start(cc_in[:, :], qblk[:, :])
                        nc.gpsimd.collective_compute(
                            "AllGather",
                            mybir.AluOpType.bypass,
                            replica_groups=[list(range(NC))],
                            ins=[cc_in.opt()],
                            outs=[cc_out.opt()],
                        )
                        nxt = sb_slots[it]
                        nc.sync.dma_start(
                            nxt[:, :].rearrange("p (d f) -> p d f", d=NC),
                            cc_out[:, :].rearrange("(d p) f -> p d f", p=128),
                        )

                        # pre-start next iteration's spatial own-slice part
                        ps_sp_pending = ps_r.tile([L, NYX], f32, tag="spat",
                                                  name=f"ps_spp{it}")
                        bo = wp.tile([128, FB], f16, tag="brto")
                        nc.vector.tensor_scalar_mul(
                            bo[:, :], qblk[:, :], sb_zcoo[:, 0:1])
                        bo_v = bo[:, :].rearrange("p (t l) -> p t l", l=L)
                        for k in range(NT):
                            for h in range(2):
                                nc.tensor.matmul(
                                    ps_sp_pending[:, h * 512:(h + 1) * 512],
                                    bo_v[:, k, :],
                                    kyx_v[:, k, h * 512:(h + 1) * 512],
                                    start=(k == 0), stop=False,
                                    skip_group_check=True,
                                )

                        ps_dmy = ps_dmy_p.tile([1, 512], f32, tag="dmy",
                                               name=f"ps_dmy{it}")
                        for _w in range(20):
                            nc.tensor.matmul(
                                ps_dmy[0:1, 0:512], sb_at[0:1, 0:1],
                                sb_kyx[0:1, 0:512],
                                start=True, stop=True,
                                skip_group_check=True,
                            )

                        slots_l = nxt[:, :].rearrange("p (n l) -> p n l", l=L)
                        slots_dt = nxt[:, :].rearrange(
                            "p (d t l) -> p d t l", d=NC, l=L)
    nc.compile()
    return nc


def _host_prep(image, logits):
    """Per-core input dicts (global voxel order). Returns list of 8 dicts."""
    img = np.asarray(image, dtype=np.float32)[0]      # [3, D, H, W]
    lg = np.asarray(logits, dtype=np.float32)[0]      # [L, D, H, W]

    zz, yy, xx = np.meshgrid(
        np.arange(D), np.arange(H), np.arange(W), indexing="ij")
    pos = np.stack([zz, yy, xx], -1).reshape(N, 3).astype(np.float64)
    rgb = img.reshape(3, N).T.astype(np.float64)
    feat = np.concatenate([pos / ALPHA, rgb / BETA], axis=1)   # [N, 6]
    sq = np.sum(feat * feat, axis=1)                           # [N]

    # quadratic fit of exp(x) over the exponent range [-xmax, 0]
    dmax = 2.0 * (31.0 / ALPHA) ** 2 + (7.0 / ALPHA) ** 2 + np.sum(
        ((rgb.max(0) - rgb.min(0)) / BETA) ** 2)
    xmax = 0.5 * dmax
    xs = np.linspace(-xmax, 0.0, 4001)
    c2, c1, c0 = np.polyfit(xs, np.exp(xs), 2)

    # u(i).v(j) = f_i.f_j - |f_i|^2/2 - |f_j|^2/2 = -|f_i - f_j|^2/2
    u = np.concatenate([feat, -0.5 * sq[:, None], np.ones((N, 1))], axis=1)
    v = np.concatenate([feat, np.ones((N, 1)), -0.5 * sq[:, None]], axis=1)
    cols = []
    pcols = []
    cols.append(np.ones(N)); pcols.append(np.full(N, c0))
    for m in range(8):
        cols.append(c1 * u[:, m]); pcols.append(v[:, m])
    for m in range(8):
        for m2 in range(m, 8):
            w = 2.0 if m2 > m else 1.0
            cols.append(w * c2 * u[:, m] * u[:, m2])
            pcols.append(v[:, m] * v[:, m2])
    PHI = np.stack(cols, axis=1).astype(np.float16)    # [N, 45]
    PSI = np.stack(pcols, axis=1).astype(np.float16)   # [N, 45]

    r1 = np.arange(D, dtype=np.float32)
    Gz = np.exp(-0.5 * ((r1[:, None] - r1[None, :]) / GAMMA) ** 2)
    r2 = np.arange(H, dtype=np.float32)
    Gy = np.exp(-0.5 * ((r2[:, None] - r2[None, :]) / GAMMA) ** 2)
    Kyx = np.kron(Gy, Gy).astype(np.float32)          # H == W so Gy == Gx
    nyx = Kyx.sum(axis=0)
    Kyx_n = (Kyx / nyx[None, :]).astype(np.float16)   # [1024, 1024]
    czsum = Gz.sum(axis=0)

    unary = lg.reshape(L, N)
    # voxel-major: blkT[p, s, t*L + l] = unary[l, s*NYX + t*128 + p]
    blkT = unary.reshape(L, D, NT, 128).transpose(3, 1, 2, 0)  # [128, D, NT, L]
    un = np.ascontiguousarray(blkT.reshape(128, NC * FB))

    phi_in = np.ascontiguousarray(
        PHI.reshape(NTILE, 128, NF).transpose(1, 0, 2).reshape(128, NTILE * NF))
    kyx_in = np.ascontiguousarray(
        Kyx_n.reshape(NT, 128, NYX).transpose(1, 0, 2).reshape(128, NT * NYX))

    maps = []
    for r in range(NC):
        psi_r = np.ascontiguousarray(PSI[r * NYX:(r + 1) * NYX].T)  # [45,1024]
        zvec = (Gz[:, r] / czsum[r]).astype(np.float32)
        zco = np.tile(zvec, (128, 1))
        zrest = zvec.copy(); zrest[r] = 0.0
        unown = np.ascontiguousarray(blkT[:, r].reshape(128, FB))
        maps.append({
            "zcoo": np.full((128, 1), zvec[r], np.float32),
            "zcor": np.ascontiguousarray(np.tile(zrest, (128, 1))),
            "phi": phi_in,
            "psi": psi_r,
            "kyx": kyx_in,
            "unaryt": un,
            "unown": unown,
            "zcoef": np.ascontiguousarray(zco),
        })
    return maps


def kernel(image, logits, spatial_ker_weights, bilateral_ker_weights,
           compatibility_matrix):
    from concourse.bass_utils import run_bass_kernel_spmd

    if "nc" not in _CACHE:
        _CACHE["nc"] = _build_nc()
    nc = _CACHE["nc"]

    maps = _host_prep(image, logits)
    ws = np.asarray(spatial_ker_weights, np.float64)
    wb = np.asarray(bilateral_ker_weights, np.float64)
    cm = np.asarray(compatibility_matrix, np.float64)
    at = np.ascontiguousarray((cm @ ws).T.astype(np.float16))
    bt = np.ascontiguousarray((cm @ wb).T.astype(np.float16))
    ones1 = np.ones((1, L), np.float32)
    for m in maps:
        m["at"] = at
        m["bt"] = bt
        m["ones1"] = ones1

    res = run_bass_kernel_spmd(nc, maps, core_ids=list(range(NC)))

    out = np.empty((L, D, H, W), dtype=np.float32)
    for r in range(NC):
        blk = res.results[r]["out"]                   # [128, 168]
        out[:, r] = blk.reshape(128, NT, L).transpose(2, 1, 0).reshape(L, H, W)
    return out[None]
